# revision 1
# baseline (speedup 1.0000x reference)
"""Trainium2 distributed kernel for the multi-query sparse-attention block.

Sharding: 8 cores = 2 batches x 4 head-groups (4 heads each).
J (key/value axis) is host-permuted to [self(2048) | ctx(256) | null(1) | pad(127)]
and the attention bias arrives pre-transposed (j-major), mask-folded and
pre-exponentiated in bf16:  attn_weight = exp(q.k) * expb.
Softmax runs without max-subtraction; the denominator comes from a ones-column
appended to V.  Output projection partials are ReduceScattered over each
4-core batch group, and the final layernorm runs on the scattered shards.
"""

import sys

sys.path.insert(0, "/opt/trn_rl_repo")

import numpy as np
import ml_dtypes

import concourse.bass as bass
import concourse.mybir as mybir
import concourse.tile as tile
from concourse import bacc
from concourse.bass_utils import run_bass_kernel_spmd
from concourse.masks import make_identity

F32 = mybir.dt.float32
F32R = mybir.dt.float32r
BF16 = mybir.dt.bfloat16
AF = mybir.ActivationFunctionType

B, N, D = 2, 2048, 1024
H, DH = 16, 64
C, CD = 256, 512
J = C + 1 + N          # 2305
JP = 19 * 128          # 2432 padded
HPC = 4                # heads per core
EPS = 1e-5

_cache = {}


def _ln_stats_multi(nc, pool, srcs, d, eps_ap):
    """Batched LN stats for k [128, d] tiles: one sqrt/recip/negmr pass.
    Returns lists (rstd_aps, negmr_aps)."""
    k = len(srcs)
    ns = d // 512
    mv = pool.tile([128, k, 2], F32, tag="lnmv")
    for j, s_ap in enumerate(srcs):
        stats = pool.tile([128, ns, 6], F32, tag="lnst")
        r = s_ap.rearrange("p (n f) -> p n f", f=512)
        for s in range(ns):
            nc.vector.bn_stats(out=stats[:, s, :], in_=r[:, s, :])
        nc.vector.bn_aggr(out=mv[:, j, :], in_=stats[:, :, :])
    rstd = pool.tile([128, k], F32, tag="lnrs")
    mvr = mv[:].rearrange("p k two -> p (k two)")
    nc.scalar.activation(rstd[:], mvr[:, 1::2], AF.Sqrt, bias=eps_ap[:, :])
    nc.vector.reciprocal(rstd[:], rstd[:])
    negmr = pool.tile([128, k], F32, tag="lnnm")
    nc.vector.tensor_tensor(out=negmr[:], in0=mvr[:, 0::2], in1=rstd[:],
                            op=mybir.AluOpType.mult)
    nc.vector.tensor_scalar_mul(out=negmr[:], in0=negmr[:], scalar1=-1.0)
    return ([rstd[:, j:j + 1] for j in range(k)],
            [negmr[:, j:j + 1] for j in range(k)])


def build():
    nc = bacc.Bacc("TRN2", target_bir_lowering=False, debug=False, num_devices=8)

    expb = nc.declare_dram_parameter("expb", [HPC, JP, N], BF16, isOutput=False)
    x_in = nc.declare_dram_parameter("x", [N, D], F32, isOutput=False)
    ctx_in = nc.declare_dram_parameter("ctxt", [C, CD], F32, isOutput=False)
    nullk = nc.declare_dram_parameter("nullk", [DH, 1], F32, isOutput=False)
    nullv = nc.declare_dram_parameter("nullv", [1, DH], F32, isOutput=False)
    wq_in = nc.declare_dram_parameter("wq", [D, 256], F32, isOutput=False)
    wkv_in = nc.declare_dram_parameter("wkv", [D, 128], F32, isOutput=False)
    wctx_in = nc.declare_dram_parameter("wctx", [CD, 128], F32, isOutput=False)
    bctx_in = nc.declare_dram_parameter("bctx2", [1, 128], F32, isOutput=False)
    wout_in = nc.declare_dram_parameter("wout", [256, D], F32, isOutput=False)
    outg_in = nc.declare_dram_parameter("outg", [1, D], F32, isOutput=False)
    out_ext = nc.declare_dram_parameter("out", [N // 4, D], F32, isOutput=True)

    rs_in = [nc.dram_tensor(f"rs_in{c}", [512, D], BF16) for c in range(4)]
    rs_out = [nc.dram_tensor(f"rs_out{c}", [128, D], BF16) for c in range(4)]

    with tile.TileContext(nc) as tc:
        with tc.tile_pool(name="persist", bufs=1) as pp:
            wq_r = pp.tile([128, 8, 256], BF16)
            wkv_r = pp.tile([128, 8, 128], BF16)
            wctx_r = pp.tile([128, 4, 128], BF16)
            wout_r = pp.tile([128, 2, 1024], F32R)
            bctx_r = pp.tile([1, 128], BF16)
            ones_r = pp.tile([1, 1024], BF16)
            ident0 = pp.tile([128, 128], F32)
            ident_r = pp.tile([128, 128], BF16)
            gamma_bc = pp.tile([128, 1024], F32)
            qT = pp.tile([64, HPC * N], BF16)
            kT = pp.tile([64, JP], BF16)
            vext = pp.tile([128, 19 * 65], BF16)
            aoT0 = pp.tile([128, N], F32R)
            aoT1 = pp.tile([128, N], F32R)
            aoT = [aoT0, aoT1]

            nc.gpsimd.dma_start(out=wq_r[:], in_=wq_in.rearrange("(c p) f -> p c f", p=128))
            nc.gpsimd.dma_start(out=wkv_r[:], in_=wkv_in.rearrange("(c p) f -> p c f", p=128))
            nc.gpsimd.dma_start(out=wctx_r[:], in_=wctx_in.rearrange("(c p) f -> p c f", p=128))
            nc.gpsimd.dma_start(out=wout_r[:], in_=wout_in.rearrange("(c p) f -> p c f", p=128))
            nc.gpsimd.dma_start(out=bctx_r[:], in_=bctx_in[:])
            nc.gpsimd.dma_start(out=kT[:, 2304:2305], in_=nullk[:])

            eps_t = pp.tile([128, 1], F32)
            nc.gpsimd.memset(eps_t[:], EPS)
            zrow = pp.tile([128, 128], F32)
            nc.vector.memset(zrow[:], 0.0)
            nc.scalar.copy(kT[:, 2305:2432], zrow[0:64, 0:127])
            nc.vector.memset(vext[:, 18 * 65:18 * 65 + 64], 0.0)
            nc.gpsimd.dma_start(out=vext[0:1, 18 * 65:18 * 65 + 64], in_=nullv[:])
            for jb in range(19):
                nc.vector.memset(vext[:, jb * 65 + 64:jb * 65 + 65], 1.0)

            o1 = pp.tile([1, 1024], F32)
            nc.vector.memset(o1[:], 1.0)
            nc.scalar.copy(ones_r[:], o1[:])
            make_identity(nc, ident0[:])
            nc.scalar.copy(ident_r[:], ident0[:])

            og_sb = pp.tile([1, 1024], F32)
            nc.sync.dma_start(out=og_sb[:], in_=outg_in[:])
            nc.gpsimd.partition_broadcast(gamma_bc[:], og_sb[:])

            # ---------------- x: LN + transpose + projections ----------------
            with tc.tile_pool(name="xt", bufs=5) as xp, \
                 tc.tile_pool(name="xst", bufs=2) as xs, \
                 tc.tile_pool(name="xnt", bufs=2) as xnp, \
                 tc.tile_pool(name="vtmp", bufs=2) as vtp, \
                 tc.tile_pool(name="xps", bufs=3, space="PSUM") as xps, \
                 tc.tile_pool(name="pps", bufs=2, space="PSUM") as pps:
                for ic in range(4):
                    xnT = xnp.tile([128, 8, 512], BF16, tag="xnT")
                    xts = []
                    for tb in range(4):
                        i0 = ic * 512 + tb * 128
                        xt = xp.tile([128, D], F32, tag="xt")
                        nc.sync.dma_start(out=xt[:], in_=x_in[i0:i0 + 128, :])
                        xts.append(xt)
                    rstds, negmrs = _ln_stats_multi(nc, xs, [t[:] for t in xts], D, eps_t)
                    for tb in range(4):
                        xn = xp.tile([128, D], BF16, tag="xn")
                        nc.scalar.activation(xn[:], xts[tb][:], AF.Identity,
                                             bias=negmrs[tb], scale=rstds[tb])
                        for ch in range(2):
                            pt = xps.tile([128, 512], BF16, tag="xtp")
                            for c in range(4):
                                cc = ch * 4 + c
                                nc.tensor.matmul(pt[:, c * 128:(c + 1) * 128],
                                                 xn[:, cc * 128:(cc + 1) * 128],
                                                 ident_r[:], is_transpose=True,
                                                 start=True, stop=True)
                            nc.scalar.copy(
                                xnT[:, ch * 4:(ch + 1) * 4, tb * 128:(tb + 1) * 128],
                                pt[:].rearrange("p (c f) -> p c f", f=128))
                    for m in range(2):
                        pq = pps.tile([128, 512], F32, tag="pq")
                        for c in range(8):
                            nc.tensor.matmul(pq[:], wq_r[:, c, m * 128:(m + 1) * 128],
                                             xnT[:, c, :],
                                             start=(c == 0), stop=(c == 7))
                        for hh in range(2):
                            h = 2 * m + hh
                            nc.scalar.copy(
                                qT[:, h * N + ic * 512:h * N + ic * 512 + 512],
                                pq[hh * 64:hh * 64 + 64, :])
                    pkv = pps.tile([128, 512], F32, tag="pkv")
                    for c in range(8):
                        nc.tensor.matmul(pkv[:], wkv_r[:, c, :], xnT[:, c, :],
                                         start=(c == 0), stop=(c == 7))
                    nc.scalar.copy(kT[:, ic * 512:ic * 512 + 512], pkv[0:64, :])
                    vt = vtp.tile([64, 512], F32, tag="vt")
                    nc.scalar.copy(vt[:], pkv[64:128, :])
                    for tb in range(4):
                        pv = xps.tile([128, 512], F32, tag="xtp")
                        nc.tensor.matmul(pv[:, 0:64], vt[:, tb * 128:(tb + 1) * 128],
                                         ident0[0:64, 0:64], is_transpose=True,
                                         start=True, stop=True)
                        jb = ic * 4 + tb
                        nc.vector.tensor_copy(vext[:, jb * 65:jb * 65 + 64], pv[:, 0:64])

            # ---------------- context tokens -> kT/vext ----------------
            with tc.tile_pool(name="cwork", bufs=2) as cw, \
                 tc.tile_pool(name="cstat", bufs=2) as cs, \
                 tc.tile_pool(name="cps", bufs=2, space="PSUM") as cps:
                cnT = pp.tile([128, 4, 256], BF16)
                cts = []
                for t in range(2):
                    ct = cw.tile([128, CD], F32, tag="ct")
                    nc.sync.dma_start(out=ct[:], in_=ctx_in[t * 128:(t + 1) * 128, :])
                    cts.append(ct)
                rstds, negmrs = _ln_stats_multi(nc, cs, [c[:] for c in cts], CD, eps_t)
                for t in range(2):
                    cn = cw.tile([128, CD], BF16, tag="cn")
                    nc.scalar.activation(cn[:], cts[t][:], AF.Identity,
                                         bias=negmrs[t], scale=rstds[t])
                    for c in range(4):
                        pt = cps.tile([128, 128], BF16, tag="ctp")
                        nc.tensor.matmul(pt[:], cn[:, c * 128:(c + 1) * 128],
                                         ident_r[:], is_transpose=True,
                                         start=True, stop=True)
                        nc.scalar.copy(cnT[:, c, t * 128:(t + 1) * 128],
                                       pt[:])
                pck = cps.tile([64, 256], F32, tag="ck")
                for c in range(4):
                    nc.tensor.matmul(pck[:], wctx_r[:, c, 0:64], cnT[:, c, :],
                                     start=(c == 0), stop=False)
                nc.tensor.matmul(pck[:], bctx_r[:, 0:64], ones_r[:, 0:256],
                                 start=False, stop=True)
                nc.scalar.copy(kT[:, 2048:2304], pck[:])
                for t in range(2):
                    pcv = cps.tile([128, 64], F32, tag="cv")
                    for c in range(4):
                        nc.tensor.matmul(pcv[:], cnT[:, c, t * 128:(t + 1) * 128],
                                         wctx_r[:, c, 64:128],
                                         start=(c == 0), stop=False)
                    nc.tensor.matmul(pcv[:], ones_r[:, 0:128], bctx_r[:, 64:128],
                                     start=False, stop=True)
                    nc.vector.tensor_copy(vext[:, (16 + t) * 65:(16 + t) * 65 + 64],
                                          pcv[:])

            # ---------------- attention + interleaved out-proj/RS/LN ----------------
            with tc.tile_pool(name="eb", bufs=8) as ebp, \
                 tc.tile_pool(name="aw", bufs=8) as awp, \
                 tc.tile_pool(name="nrm", bufs=2) as nrm, \
                 tc.tile_pool(name="ysb", bufs=3) as yp, \
                 tc.tile_pool(name="fst", bufs=2) as fs, \
                 tc.tile_pool(name="aps", bufs=6, space="PSUM") as aps, \
                 tc.tile_pool(name="ops", bufs=2, space="PSUM") as ops:

                def out_block(ib):
                    # one 128-token block of the output projection
                    y = yp.tile([128, 1024], BF16, tag="y")
                    for ec in range(2):
                        py = aps.tile([128, 512], F32, tag="ps")
                        for c in range(2):
                            nc.tensor.matmul(py[:],
                                             aoT[c][:, ib * 128:(ib + 1) * 128],
                                             wout_r[:, c, ec * 512:(ec + 1) * 512],
                                             start=(c == 0), stop=(c == 1))
                        if ec == 0:
                            nc.vector.tensor_copy(y[:, 0:512], py[:])
                        else:
                            nc.scalar.copy(y[:, 512:1024], py[:])
                    ch = ib // 4
                    nc.sync.dma_start(
                        out=rs_in[ch][(ib % 4) * 128:(ib % 4 + 1) * 128, :], in_=y[:])

                def issue_rs(ch):
                    nc.gpsimd.collective_compute(
                        "ReduceScatter", mybir.AluOpType.add,
                        replica_groups=[[0, 1, 2, 3], [4, 5, 6, 7]],
                        ins=[rs_in[ch][:]], outs=[rs_out[ch][:]])

                def final_ln(ch):
                    ft = yp.tile([128, 1024], F32, tag="ft")
                    nc.gpsimd.dma_start(out=ft[:], in_=rs_out[ch][:])
                    rstds, negmrs = _ln_stats_multi(nc, fs, [ft[:]], D, eps_t)
                    fn = yp.tile([128, 1024], F32, tag="fn")
                    nc.scalar.activation(fn[:], ft[:], AF.Identity,
                                         bias=negmrs[0], scale=rstds[0])
                    nc.vector.tensor_mul(fn[:], fn[:], gamma_bc[:])
                    nc.gpsimd.dma_start(out=out_ext[ch * 128:(ch + 1) * 128, :],
                                      in_=fn[:])

                for iq in range(4):
                    for h in range(HPC):
                        po = ops.tile([65, 512], F32, tag="po")
                        aws = {}

                        def emit_sim(jb):
                            ps = aps.tile([128, 512], F32, tag="ps", name=f"ps{jb}")
                            eb = ebp.tile([128, 512], BF16, tag="eb", name=f"eb{jb}")
                            nc.sync.dma_start(
                                out=eb[:],
                                in_=expb[h, jb * 128:(jb + 1) * 128,
                                         iq * 512:(iq + 1) * 512])
                            nc.tensor.matmul(
                                ps[:],
                                kT[:, jb * 128:(jb + 1) * 128],
                                qT[:, h * N + iq * 512:h * N + iq * 512 + 512],
                                start=True, stop=True)
                            et = awp.tile([128, 512], BF16, tag="et", name=f"et{jb}")
                            nc.scalar.activation(et[:], ps[:], AF.Exp)
                            aw = awp.tile([128, 512], BF16, tag="aw", name=f"aw{jb}")
                            nc.vector.tensor_mul(aw[:], et[:], eb[:])
                            aws[jb] = aw

                        def emit_av(jb):
                            nc.tensor.matmul(
                                po[:],
                                vext[:, jb * 65:jb * 65 + 65],
                                aws.pop(jb)[:],
                                start=(jb == 0), stop=(jb == 18))

                        # pair-grouped software pipeline, attnV 4 behind sim
                        for jb0 in range(0, 24, 2):
                            for jb in (jb0, jb0 + 1):
                                if jb < 19:
                                    emit_sim(jb)
                            for jb in (jb0 - 4, jb0 - 3):
                                if 0 <= jb < 19:
                                    emit_av(jb)
                        lg = nrm.tile([1, 512], F32, tag="lg")
                        nc.scalar.activation(lg[:], po[64:65, :], AF.Ln)
                        rec = nrm.tile([1, 512], F32, tag="rec")
                        nc.scalar.activation(rec[:], lg[:], AF.Exp, scale=-1.0)
                        rbc = nrm.tile([64, 512], F32, tag="rbc")
                        nc.gpsimd.partition_broadcast(rbc[:], rec[:])
                        nc.vector.tensor_mul(
                            aoT[h // 2][(h % 2) * 64:(h % 2) * 64 + 64,
                                        iq * 512:(iq + 1) * 512],
                            po[0:64, :], rbc[:])
                    for ibl in range(4):
                        out_block(iq * 4 + ibl)
                    issue_rs(iq)
                for ch in range(4):
                    final_ln(ch)

    nc.compile()
    return nc


def _prep(inputs):
    x = np.asarray(inputs["x"], dtype=np.float32)
    context = np.asarray(inputs["context"], dtype=np.float32)
    mask = np.asarray(inputs["mask"])
    ab = np.asarray(inputs["attn_bias"], dtype=np.float32)
    norm_gamma = np.asarray(inputs["norm_gamma"], dtype=np.float32)
    null_kv = np.asarray(inputs["null_kv"], dtype=np.float32)
    Wq = np.asarray(inputs["Wq"], dtype=np.float32)
    Wkv = np.asarray(inputs["Wkv"], dtype=np.float32)
    ctx_ln_w = np.asarray(inputs["ctx_ln_w"], dtype=np.float32)
    ctx_ln_b = np.asarray(inputs["ctx_ln_b"], dtype=np.float32)
    Wctx = np.asarray(inputs["Wctx"], dtype=np.float32)
    bctx = np.asarray(inputs["bctx"], dtype=np.float32)
    Wout = np.asarray(inputs["Wout"], dtype=np.float32)
    out_gamma = np.asarray(inputs["out_gamma"], dtype=np.float32)

    scale = DH ** -0.5
    wq_f = (norm_gamma[:, None] * Wq) * scale            # (D, H*DH)
    wkv_f = np.ascontiguousarray(norm_gamma[:, None] * Wkv)
    wctx_f = np.ascontiguousarray(ctx_ln_w[:, None] * Wctx)
    bctx2 = np.ascontiguousarray((ctx_ln_b @ Wctx + bctx)[None, :])
    outg = np.ascontiguousarray(out_gamma[None, :])
    nullk = np.ascontiguousarray(null_kv[0][:, None])
    nullv = np.ascontiguousarray(null_kv[1][None, :])

    # J permute [self | ctx | null], transpose j-major, exponentiate
    bp = np.concatenate([ab[:, :, C + 1:], ab[:, :, :C + 1]], axis=2)
    ebT = np.exp(np.ascontiguousarray(bp.transpose(0, 2, 1)))  # (H, J, N) f32
    mvec = np.empty((B, J), dtype=np.float32)
    mvec[:, :N] = mask[:, C:]
    mvec[:, N] = 1.0                       # ctx[0]: the left-pad True
    mvec[:, N + 1:N + C] = mask[:, :C - 1]  # ctx[c] <- mask[c-1]
    mvec[:, N + C] = mask[:, C - 1]         # null <- mask[255]

    in_maps = []
    for core in range(8):
        b, g = core // 4, core % 4
        eb = ebT[HPC * g:HPC * g + HPC] * mvec[b][None, :, None]
        ebp = np.zeros((HPC, JP, N), dtype=ml_dtypes.bfloat16)
        ebp[:, :J, :] = eb.astype(ml_dtypes.bfloat16)
        in_maps.append({
            "expb": ebp,
            "x": np.ascontiguousarray(x[b]),
            "ctxt": np.ascontiguousarray(context[b]),
            "nullk": nullk,
            "nullv": nullv,
            "wq": np.ascontiguousarray(wq_f[:, 256 * g:256 * (g + 1)]),
            "wkv": wkv_f,
            "wctx": wctx_f,
            "bctx2": bctx2,
            "wout": np.ascontiguousarray(Wout[256 * g:256 * (g + 1), :]),
            "outg": outg,
        })
    return in_maps


def kernel(**inputs) -> np.ndarray:
    if "nc" not in _cache:
        _cache["nc"] = build()
    nc = _cache["nc"]
    in_maps = _prep(inputs)
    res = run_bass_kernel_spmd(nc, in_maps, core_ids=list(range(8))).results
    out = np.empty((B, N, D), dtype=np.float32)
    for core in range(8):
        b, r = core // 4, core % 4
        o = res[core]["out"]
        for ch in range(4):
            out[b, 512 * ch + 128 * r:512 * ch + 128 * (r + 1), :] = \
                o[ch * 128:(ch + 1) * 128]
    return out



# revision 3
# speedup vs baseline: 1.1696x; 1.1696x over previous
"""Trainium2 distributed kernel for the multi-query sparse-attention block.

Sharding: 8 cores = 2 batches x 4 head-groups (4 heads each).
J (key/value axis) is host-permuted to [self(2048) | ctx(256) | null(1) | pad(127)]
and the attention bias arrives pre-transposed (j-major), mask-folded and
pre-exponentiated in bf16:  attn_weight = exp(q.k) * expb.

Attention processes HEAD PAIRS: head A lives on SBUF partitions 0-63, head B
on 64-127 (kT/qT duplicated/stacked), so the two qk matmuls run CONCURRENTLY
on the PE's two 64-row tiles (tile_position (0,0) / (64,0)).  The two sim
tiles land in adjacent PSUM banks and are exponentiated by a single 1024-wide
scalar-engine instruction (the scalar engine exp stream is the pacing
resource).  Softmax runs without max-subtraction; denominators come from a
ones-column appended to V and are reciprocated on the vector engine.  All
layernorm rstds use exp(-0.5*ln(var+eps)) so the scalar engine never switches
activation tables.  Output projection partials are ReduceScattered over each
4-core batch group; final layernorms are pipelined into the attention loop.
"""

import sys

sys.path.insert(0, "/opt/trn_rl_repo")

import numpy as np
import ml_dtypes

import concourse.bass as bass
import concourse.mybir as mybir
import concourse.tile as tile
from concourse import bacc
from concourse.bass_utils import run_bass_kernel_spmd
from concourse.masks import make_identity

F32 = mybir.dt.float32
F32R = mybir.dt.float32r
BF16 = mybir.dt.bfloat16
AF = mybir.ActivationFunctionType
ALU = mybir.AluOpType

B, N, D = 2, 2048, 1024
H, DH = 16, 64
C, CD = 256, 512
J = C + 1 + N          # 2305
JP = 19 * 128          # 2432 padded
HPC = 4                # heads per core
EPS = 1e-5

_cache = {}


def _ln_stats(nc, pool, srcs, d, eps_ap):
    """Batched LN stats for k [128, d] tiles.  rstd = exp(-0.5*ln(var+eps))
    (stays on the ln/exp activation table - no table switch).
    Returns (rstd_aps, mean_aps, negmr_aps)."""
    k = len(srcs)
    ns = d // 512
    mv = pool.tile([128, k, 2], F32, tag="lnmv")
    for j, s_ap in enumerate(srcs):
        stats = pool.tile([128, ns, 6], F32, tag="lnst")
        r = s_ap.rearrange("p (n f) -> p n f", f=512)
        for s in range(ns):
            nc.vector.bn_stats(out=stats[:, s, :], in_=r[:, s, :])
        nc.vector.bn_aggr(out=mv[:, j, :], in_=stats[:, :, :])
    mvr = mv[:].rearrange("p k two -> p (k two)")
    lnv = pool.tile([128, k], F32, tag="lnlv")
    nc.scalar.activation(lnv[:], mvr[:, 1::2], AF.Ln, bias=eps_ap[:, :])
    rstd = pool.tile([128, k], F32, tag="lnrs")
    nc.scalar.activation(rstd[:], lnv[:], AF.Exp, scale=-0.5)
    negmr = pool.tile([128, k], F32, tag="lnnm")
    nc.vector.scalar_tensor_tensor(
        out=negmr[:], in0=mvr[:, 0::2], scalar=-1.0, in1=rstd[:],
        op0=ALU.mult, op1=ALU.mult)
    return ([rstd[:, j:j + 1] for j in range(k)],
            [mvr[:, 2 * j:2 * j + 1] for j in range(k)],
            [negmr[:, j:j + 1] for j in range(k)])


def build():
    nc = bacc.Bacc("TRN2", target_bir_lowering=False, debug=False, num_devices=8)

    expb = nc.declare_dram_parameter("expb", [HPC, JP, N], BF16, isOutput=False)
    x_in = nc.declare_dram_parameter("x", [N, D], F32, isOutput=False)
    ctx_in = nc.declare_dram_parameter("ctxt", [C, CD], F32, isOutput=False)
    nullk = nc.declare_dram_parameter("nullk", [128, 1], F32, isOutput=False)
    nullv = nc.declare_dram_parameter("nullv", [1, DH], F32, isOutput=False)
    wq_in = nc.declare_dram_parameter("wq", [D, 256], F32, isOutput=False)
    wkv_in = nc.declare_dram_parameter("wkv", [D, 128], F32, isOutput=False)
    wctx_in = nc.declare_dram_parameter("wctx", [CD, 128], F32, isOutput=False)
    bctx_in = nc.declare_dram_parameter("bctx2", [1, 128], F32, isOutput=False)
    wout_in = nc.declare_dram_parameter("wout", [256, D], F32, isOutput=False)
    outg_in = nc.declare_dram_parameter("outg", [1, D], F32, isOutput=False)
    out_ext = nc.declare_dram_parameter("out", [N // 4, D], F32, isOutput=True)

    rs_in = [nc.dram_tensor(f"rs_in{c}", [512, D], BF16) for c in range(4)]
    rs_out = [nc.dram_tensor(f"rs_out{c}", [128, D], BF16) for c in range(4)]

    with tile.TileContext(nc) as tc:
        with tc.tile_pool(name="persist", bufs=1) as pp:
            wq_r = pp.tile([128, 8, 256], BF16)
            wkv_r = pp.tile([128, 8, 128], BF16)
            wctx_r = pp.tile([128, 4, 128], BF16)
            wout_r = pp.tile([128, 2, 1024], F32R)
            bctx_r = pp.tile([1, 128], BF16)
            ones_r = pp.tile([1, 1024], BF16)
            ident0 = pp.tile([128, 128], F32)
            ident_r = pp.tile([128, 128], BF16)
            gamma_bc = pp.tile([128, 1024], F32)
            qT2 = pp.tile([128, 2, N], BF16)     # [pairstack, m, n]
            kT = pp.tile([128, JP], BF16)        # both halves hold same kT
            vext = pp.tile([128, 19 * 65], BF16)
            aoT0 = pp.tile([128, N], F32R)
            aoT1 = pp.tile([128, N], F32R)
            aoT = [aoT0, aoT1]

            nc.gpsimd.dma_start(out=wq_r[:], in_=wq_in.rearrange("(c p) f -> p c f", p=128))
            nc.gpsimd.dma_start(out=wkv_r[:], in_=wkv_in.rearrange("(c p) f -> p c f", p=128))
            nc.gpsimd.dma_start(out=wctx_r[:], in_=wctx_in.rearrange("(c p) f -> p c f", p=128))
            nc.gpsimd.dma_start(out=wout_r[:], in_=wout_in.rearrange("(c p) f -> p c f", p=128))
            nc.gpsimd.dma_start(out=bctx_r[:], in_=bctx_in[:])
            nc.gpsimd.dma_start(out=kT[:, 2304:2305], in_=nullk[:])

            eps_t = pp.tile([128, 1], F32)
            nc.gpsimd.memset(eps_t[:], EPS)
            nc.vector.memset(kT[:, 2305:2432], 0.0)
            nc.vector.memset(vext[:, 18 * 65:18 * 65 + 64], 0.0)
            nc.gpsimd.dma_start(out=vext[0:1, 18 * 65:18 * 65 + 64], in_=nullv[:])
            for jb in range(19):
                nc.vector.memset(vext[:, jb * 65 + 64:jb * 65 + 65], 1.0)

            o1 = pp.tile([1, 1024], F32)
            nc.vector.memset(o1[:], 1.0)
            nc.scalar.copy(ones_r[:], o1[:])
            make_identity(nc, ident0[:])
            nc.scalar.copy(ident_r[:], ident0[:])

            og_sb = pp.tile([1, 1024], F32)
            nc.sync.dma_start(out=og_sb[:], in_=outg_in[:])
            nc.gpsimd.partition_broadcast(gamma_bc[:], og_sb[:])

            # ---------------- x: LN + transpose + projections ----------------
            with tc.tile_pool(name="xt", bufs=8) as xp, \
                 tc.tile_pool(name="xst", bufs=2) as xs, \
                 tc.tile_pool(name="xnt", bufs=2) as xnp, \
                 tc.tile_pool(name="vtmp", bufs=2) as vtp, \
                 tc.tile_pool(name="xps", bufs=3, space="PSUM") as xps, \
                 tc.tile_pool(name="pps", bufs=2, space="PSUM") as pps:
                for ic in range(4):
                    xnT = xnp.tile([128, 8, 512], BF16, tag="xnT")
                    xts = []
                    for tb in range(4):
                        i0 = ic * 512 + tb * 128
                        xt = xp.tile([128, D], F32, tag="xt")
                        nc.sync.dma_start(out=xt[:], in_=x_in[i0:i0 + 128, :])
                        xts.append(xt)
                    rstds, means, negmrs = _ln_stats(
                        nc, xs, [t[:] for t in xts], D, eps_t)
                    for tb in range(4):
                        xn = xp.tile([128, D], BF16, tag="xn")
                        if tb % 2 == 0:
                            nc.scalar.activation(xn[:], xts[tb][:], AF.Identity,
                                                 bias=negmrs[tb], scale=rstds[tb])
                        else:
                            nc.vector.tensor_scalar(
                                out=xn[:], in0=xts[tb][:],
                                scalar1=means[tb], scalar2=rstds[tb],
                                op0=ALU.subtract, op1=ALU.mult)
                        for ch in range(2):
                            pt = xps.tile([128, 512], BF16, tag="xtp")
                            for c in range(4):
                                cc = ch * 4 + c
                                nc.tensor.matmul(pt[:, c * 128:(c + 1) * 128],
                                                 xn[:, cc * 128:(cc + 1) * 128],
                                                 ident_r[:], is_transpose=True,
                                                 start=True, stop=True)
                            dst = xnT[:, ch * 4:(ch + 1) * 4, tb * 128:(tb + 1) * 128]
                            src = pt[:].rearrange("p (c f) -> p c f", f=128)
                            if ch == 0:
                                nc.scalar.copy(dst, src)
                            else:
                                nc.vector.tensor_copy(dst, src)
                    for m in range(2):
                        pq = pps.tile([128, 512], F32, tag="pq")
                        for c in range(8):
                            nc.tensor.matmul(pq[:], wq_r[:, c, m * 128:(m + 1) * 128],
                                             xnT[:, c, :],
                                             start=(c == 0), stop=(c == 7))
                        nc.vector.tensor_copy(
                            qT2[:, m, ic * 512:ic * 512 + 512], pq[:])
                    pkv = pps.tile([128, 512], F32, tag="pkv")
                    for c in range(8):
                        nc.tensor.matmul(pkv[:], wkv_r[:, c, :], xnT[:, c, :],
                                         start=(c == 0), stop=(c == 7))
                    nc.scalar.copy(kT[0:64, ic * 512:ic * 512 + 512], pkv[0:64, :])
                    nc.scalar.copy(kT[64:128, ic * 512:ic * 512 + 512], pkv[0:64, :])
                    vt = vtp.tile([64, 512], F32, tag="vt")
                    nc.scalar.copy(vt[:], pkv[64:128, :])
                    for tb in range(4):
                        pv = xps.tile([128, 512], F32, tag="xtp")
                        nc.tensor.matmul(pv[:, 0:64], vt[:, tb * 128:(tb + 1) * 128],
                                         ident0[0:64, 0:64], is_transpose=True,
                                         start=True, stop=True)
                        jb = ic * 4 + tb
                        nc.vector.tensor_copy(vext[:, jb * 65:jb * 65 + 64], pv[:, 0:64])

            # ---------------- context tokens -> kT/vext ----------------
            with tc.tile_pool(name="cwork", bufs=2) as cw, \
                 tc.tile_pool(name="cstat", bufs=2) as cs, \
                 tc.tile_pool(name="cps", bufs=2, space="PSUM") as cps:
                cnT = pp.tile([128, 4, 256], BF16)
                cts = []
                for t in range(2):
                    ct = cw.tile([128, CD], F32, tag="ct")
                    nc.sync.dma_start(out=ct[:], in_=ctx_in[t * 128:(t + 1) * 128, :])
                    cts.append(ct)
                rstds, means, negmrs = _ln_stats(
                    nc, cs, [c[:] for c in cts], CD, eps_t)
                for t in range(2):
                    cn = cw.tile([128, CD], BF16, tag="cn")
                    nc.scalar.activation(cn[:], cts[t][:], AF.Identity,
                                         bias=negmrs[t], scale=rstds[t])
                    for c in range(4):
                        pt = cps.tile([128, 128], BF16, tag="ctp")
                        nc.tensor.matmul(pt[:], cn[:, c * 128:(c + 1) * 128],
                                         ident_r[:], is_transpose=True,
                                         start=True, stop=True)
                        nc.scalar.copy(cnT[:, c, t * 128:(t + 1) * 128],
                                       pt[:])
                pck = cps.tile([64, 256], F32, tag="ck")
                for c in range(4):
                    nc.tensor.matmul(pck[:], wctx_r[:, c, 0:64], cnT[:, c, :],
                                     start=(c == 0), stop=False)
                nc.tensor.matmul(pck[:], bctx_r[:, 0:64], ones_r[:, 0:256],
                                 start=False, stop=True)
                nc.scalar.copy(kT[0:64, 2048:2304], pck[:])
                nc.scalar.copy(kT[64:128, 2048:2304], pck[:])
                for t in range(2):
                    pcv = cps.tile([128, 64], F32, tag="cv")
                    for c in range(4):
                        nc.tensor.matmul(pcv[:], cnT[:, c, t * 128:(t + 1) * 128],
                                         wctx_r[:, c, 64:128],
                                         start=(c == 0), stop=False)
                    nc.tensor.matmul(pcv[:], ones_r[:, 0:128], bctx_r[:, 64:128],
                                     start=False, stop=True)
                    nc.vector.tensor_copy(vext[:, (16 + t) * 65:(16 + t) * 65 + 64],
                                          pcv[:])

            # ------------- attention + interleaved out-proj/RS/LN -------------
            with tc.tile_pool(name="eb", bufs=6) as ebp, \
                 tc.tile_pool(name="aw", bufs=4) as awp, \
                 tc.tile_pool(name="et", bufs=3) as etp, \
                 tc.tile_pool(name="nrm", bufs=2) as nrm, \
                 tc.tile_pool(name="ysb", bufs=3) as yp, \
                 tc.tile_pool(name="fst", bufs=2) as fs, \
                 tc.tile_pool(name="aps", bufs=2, space="PSUM") as aps, \
                 tc.tile_pool(name="pops", bufs=1, space="PSUM") as pops, \
                 tc.tile_pool(name="ops", bufs=2, space="PSUM") as ops:

                def pair_block(iq, m):
                    poA = pops.tile([65, 512], F32, tag="poA", name=f"poA{iq}{m}")
                    poB = pops.tile([65, 512], F32, tag="poB", name=f"poB{iq}{m}")
                    q0 = iq * 512
                    aws = {}

                    def emit_sim(jb):
                        eb = ebp.tile([128, 2, 512], BF16, tag="eb", name=f"eb{jb}")
                        nc.sync.dma_start(
                            out=eb[:],
                            in_=expb[2 * m:2 * m + 2, jb * 128:(jb + 1) * 128,
                                     q0:q0 + 512].rearrange("h p f -> p h f"))
                        ps = aps.tile([128, 1024], F32, tag="ps", name=f"ps{jb}")
                        nc.tensor.matmul(ps[:, 0:512],
                                         kT[0:64, jb * 128:(jb + 1) * 128],
                                         qT2[0:64, m, q0:q0 + 512],
                                         start=True, stop=True)
                        nc.tensor.matmul(ps[:, 512:1024],
                                         kT[64:128, jb * 128:(jb + 1) * 128],
                                         qT2[64:128, m, q0:q0 + 512],
                                         start=True, stop=True)
                        et = etp.tile([128, 1024], BF16, tag="et", name=f"et{jb}")
                        nc.scalar.activation(et[:], ps[:], AF.Exp)
                        aw = awp.tile([128, 1024], BF16, tag="aw", name=f"aw{jb}")
                        nc.vector.tensor_mul(
                            aw[:], et[:], eb[:].rearrange("p h f -> p (h f)"))
                        aws[jb] = aw

                    def emit_av(jb):
                        aw = aws.pop(jb)
                        nc.tensor.matmul(poA[:], vext[:, jb * 65:jb * 65 + 65],
                                         aw[:, 0:512],
                                         start=(jb == 0), stop=(jb == 18))
                        nc.tensor.matmul(poB[:], vext[:, jb * 65:jb * 65 + 65],
                                         aw[:, 512:1024],
                                         start=(jb == 0), stop=(jb == 18))

                    for jb in range(19):
                        emit_sim(jb)
                        if jb >= 2:
                            emit_av(jb - 2)
                    emit_av(17)
                    emit_av(18)

                    for hh, po in ((0, poA), (1, poB)):
                        rec = nrm.tile([1, 512], F32, tag="rec", name=f"rec{hh}")
                        nc.vector.reciprocal(rec[:], po[64:65, :])
                        rbc = nrm.tile([64, 512], F32, tag="rbc", name=f"rbc{hh}")
                        nc.gpsimd.partition_broadcast(rbc[:], rec[:])
                        nc.vector.tensor_mul(
                            aoT[m][hh * 64:hh * 64 + 64, q0:q0 + 512],
                            po[0:64, :], rbc[:])

                def out_block(ib):
                    y = yp.tile([128, 1024], BF16, tag="y")
                    for ec in range(2):
                        py = ops.tile([128, 512], F32, tag="py")
                        for c in range(2):
                            nc.tensor.matmul(py[:],
                                             aoT[c][:, ib * 128:(ib + 1) * 128],
                                             wout_r[:, c, ec * 512:(ec + 1) * 512],
                                             start=(c == 0), stop=(c == 1))
                        if ec == 0:
                            nc.vector.tensor_copy(y[:, 0:512], py[:])
                        else:
                            nc.scalar.copy(y[:, 512:1024], py[:])
                    ch = ib // 4
                    nc.sync.dma_start(
                        out=rs_in[ch][(ib % 4) * 128:(ib % 4 + 1) * 128, :], in_=y[:])

                def issue_rs(ch):
                    nc.gpsimd.collective_compute(
                        "ReduceScatter", mybir.AluOpType.add,
                        replica_groups=[[0, 1, 2, 3], [4, 5, 6, 7]],
                        ins=[rs_in[ch][:]], outs=[rs_out[ch][:]])

                def final_ln(ch):
                    ft = yp.tile([128, 1024], F32, tag="ft")
                    nc.gpsimd.dma_start(out=ft[:], in_=rs_out[ch][:])
                    rstds, means, _ = _ln_stats(nc, fs, [ft[:]], D, eps_t)
                    fn = yp.tile([128, 1024], F32, tag="fn")
                    nc.vector.tensor_scalar(
                        out=fn[:], in0=ft[:], scalar1=means[0], scalar2=rstds[0],
                        op0=ALU.subtract, op1=ALU.mult)
                    nc.vector.tensor_mul(fn[:], fn[:], gamma_bc[:])
                    nc.gpsimd.dma_start(out=out_ext[ch * 128:(ch + 1) * 128, :],
                                        in_=fn[:])

                for iq in range(4):
                    for m in range(2):
                        pair_block(iq, m)
                    if iq >= 1:
                        final_ln(iq - 1)
                    for ibl in range(4):
                        out_block(iq * 4 + ibl)
                    issue_rs(iq)
                final_ln(3)

    nc.compile()
    return nc


def _prep(inputs):
    x = np.asarray(inputs["x"], dtype=np.float32)
    context = np.asarray(inputs["context"], dtype=np.float32)
    mask = np.asarray(inputs["mask"])
    ab = np.asarray(inputs["attn_bias"], dtype=np.float32)
    norm_gamma = np.asarray(inputs["norm_gamma"], dtype=np.float32)
    null_kv = np.asarray(inputs["null_kv"], dtype=np.float32)
    Wq = np.asarray(inputs["Wq"], dtype=np.float32)
    Wkv = np.asarray(inputs["Wkv"], dtype=np.float32)
    ctx_ln_w = np.asarray(inputs["ctx_ln_w"], dtype=np.float32)
    ctx_ln_b = np.asarray(inputs["ctx_ln_b"], dtype=np.float32)
    Wctx = np.asarray(inputs["Wctx"], dtype=np.float32)
    bctx = np.asarray(inputs["bctx"], dtype=np.float32)
    Wout = np.asarray(inputs["Wout"], dtype=np.float32)
    out_gamma = np.asarray(inputs["out_gamma"], dtype=np.float32)

    scale = DH ** -0.5
    wq_f = (norm_gamma[:, None] * Wq) * scale            # (D, H*DH)
    wkv_f = np.ascontiguousarray(norm_gamma[:, None] * Wkv)
    wctx_f = np.ascontiguousarray(ctx_ln_w[:, None] * Wctx)
    bctx2 = np.ascontiguousarray((ctx_ln_b @ Wctx + bctx)[None, :])
    outg = np.ascontiguousarray(out_gamma[None, :])
    nullk = np.ascontiguousarray(np.tile(null_kv[0][:, None], (2, 1)))  # [128,1]
    nullv = np.ascontiguousarray(null_kv[1][None, :])

    # J permute [self | ctx | null], transpose j-major, exponentiate
    bp = np.concatenate([ab[:, :, C + 1:], ab[:, :, :C + 1]], axis=2)
    ebT = np.exp(np.ascontiguousarray(bp.transpose(0, 2, 1)))  # (H, J, N) f32
    mvec = np.empty((B, J), dtype=np.float32)
    mvec[:, :N] = mask[:, C:]
    mvec[:, N] = 1.0                       # ctx[0]: the left-pad True
    mvec[:, N + 1:N + C] = mask[:, :C - 1]  # ctx[c] <- mask[c-1]
    mvec[:, N + C] = mask[:, C - 1]         # null <- mask[255]

    in_maps = []
    for core in range(8):
        b, g = core // 4, core % 4
        eb = ebT[HPC * g:HPC * g + HPC] * mvec[b][None, :, None]
        ebp = np.zeros((HPC, JP, N), dtype=ml_dtypes.bfloat16)
        ebp[:, :J, :] = eb.astype(ml_dtypes.bfloat16)
        in_maps.append({
            "expb": ebp,
            "x": np.ascontiguousarray(x[b]),
            "ctxt": np.ascontiguousarray(context[b]),
            "nullk": nullk,
            "nullv": nullv,
            "wq": np.ascontiguousarray(wq_f[:, 256 * g:256 * (g + 1)]),
            "wkv": wkv_f,
            "wctx": wctx_f,
            "bctx2": bctx2,
            "wout": np.ascontiguousarray(Wout[256 * g:256 * (g + 1), :]),
            "outg": outg,
        })
    return in_maps


def kernel(**inputs) -> np.ndarray:
    if "nc" not in _cache:
        _cache["nc"] = build()
    nc = _cache["nc"]
    in_maps = _prep(inputs)
    res = run_bass_kernel_spmd(nc, in_maps, core_ids=list(range(8))).results
    out = np.empty((B, N, D), dtype=np.float32)
    for core in range(8):
        b, r = core // 4, core % 4
        o = res[core]["out"]
        for ch in range(4):
            out[b, 512 * ch + 128 * r:512 * ch + 128 * (r + 1), :] = \
                o[ch * 128:(ch + 1) * 128]
    return out
